# revision 22
# baseline (speedup 1.0000x reference)
"""Trainium2 Bass kernel for nn_AlignerOT: per-sample entropic Sinkhorn OT aligner.

reference math:
    src = X @ W.T + b                                   [N, D]
    cost[i,j,k] = (src[i,k] - Y[i,j])^2 * SCALE         [N, D, D]
    P[i] = sinkhorn_log(cost[i], uniform, uniform)      50 iters, eps=0.1
    ot = P.mean(0) * D * SCALE + delta_ot
    out = src @ ot

Device algorithm (math-equivalent, per sample i):
    gamma = SCALE/eps = 3000; cross_jk = 2*gamma*Y_j*src_k; s2_k = gamma*src_k^2
    pre_jk = cross_jk - s2_k + BC_k  (rank-1 fp16-split matmuls into PSUM:
             cross = yh*sh + yh*sl + yl*sh reproduces the f32 product to ~1e-2
             absolute, validated end-to-end at 7e-4 rel err)
    The log-domain Sinkhorn trajectory is reproduced exactly by kernel-space
    iterations u <- 1/(M v), v <- 1/(M^T u) on M = exp(pre + BR_j),
    restabilized every R=10 iterations by absorbing ln u, ln v into the
    row/col biases BR, BC and rebuilding M (the dual potentials span
    thousands of decades; segment-chaining is exact).  50 iters = 5 seg x 10.
    MT (the [k,j] layout needed by the u-side matvec) is an exact bf16 DMA
    transpose of M.  The two group samples' matvecs run on col-groups 0 and
    32 of the PE array concurrently (measured 2.0x).
    P[i]*D = exp(pre + BR) after the final absorb.
Sharding: data-parallel over N (16 samples/core); AllReduce sum_i P[i]*D;
    ot = (SCALE/N)*AR + delta; out rows per-core; host concat.
"""

import numpy as np

N_CORES = 8
N_GLOB = 128
NS = N_GLOB // N_CORES   # 16 samples per core
S_IN = 768
SC = S_IN // 128         # 6
D = 1024
JC = D // 128            # 8
EPS = 0.1
SCALE = 300.0
GAMMA = SCALE / EPS
RT2G = float(np.sqrt(2.0 * GAMMA))
RTG = float(np.sqrt(GAMMA))
N_ITERS = 50
N_SEG = 5
GROUP = 2

_cache = {}


def build(n_iters=N_ITERS, n_seg=N_SEG, ns=NS, group=GROUP, n_cores=N_CORES,
          skip_collective=False):
    import concourse.bass as bass
    import concourse.bacc as bacc
    import concourse.tile as tile
    import concourse.mybir as mybir
    from concourse.masks import make_identity

    fp32 = mybir.dt.float32
    bf16 = mybir.dt.bfloat16
    fp16 = mybir.dt.float16
    AF = mybir.ActivationFunctionType
    ALU = mybir.AluOpType
    AX = mybir.AxisListType
    ET = mybir.EngineType

    nc = bacc.Bacc("TRN2", target_bir_lowering=False, debug=False,
                   num_devices=n_cores)

    x_d = nc.dram_tensor("x", [ns, S_IN], fp32, kind="ExternalInput")
    y_d = nc.dram_tensor("y", [ns, D], fp32, kind="ExternalInput")
    w_d = nc.dram_tensor("w", [D, S_IN], fp32, kind="ExternalInput")
    b_d = nc.dram_tensor("bvec", [1, D], fp32, kind="ExternalInput")
    delta_d = nc.dram_tensor("delta", [D, D], fp32, kind="ExternalInput")
    out_d = nc.dram_tensor("out", [ns, D], fp32, kind="ExternalOutput")

    assert ns % group == 0
    assert n_iters % n_seg == 0
    seg_len = n_iters // n_seg

    # packed per-group-slot column vectors inside svf [128, 64] f32:
    UCF, VCF, BRC, BCC, LNX, S2C, AUXC = (
        slice(0, 8), slice(8, 16), slice(16, 24), slice(24, 32),
        slice(32, 40), slice(40, 48), slice(48, 56))
    RM = slice(56, 58)

    with tile.TileContext(nc) as tc:
        with (
            tc.tile_pool(name="const", bufs=1) as cpool,
            tc.tile_pool(name="rdata", bufs=1) as rpool,
            tc.tile_pool(name="acc", bufs=1) as apool,
            tc.tile_pool(name="ps_b", bufs=2, space="PSUM") as ps_b,
            tc.tile_pool(name="ps_i", bufs=3, space="PSUM") as ps_i,
            tc.tile_pool(name="dram", bufs=2, space="DRAM") as dpool,
        ):
            neg1h = cpool.tile([1, 128], fp16)
            nc.gpsimd.memset(neg1h[:], -1.0)

            # ---------------- phase 1: src = X @ W.T + b --------------------
            src_sb = rpool.tile([ns, D], fp32)
            y_sb = rpool.tile([ns, D], fp32)
            nc.sync.dma_start(y_sb[:], y_d.ap()[:])
            srccol = rpool.tile([128, ns, JC], fp32)
            with tc.tile_pool(name="wls", bufs=1) as wpool:
                identf = wpool.tile([128, 128], fp32)
                make_identity(nc, identf[:])
                xt = wpool.tile([128, SC, ns], fp32)
                for sc in range(SC):
                    nc.sync.dma_start(
                        xt[:, sc, :],
                        x_d.ap()[:, sc * 128:(sc + 1) * 128].rearrange(
                            "n p -> p n"))
                ones16 = wpool.tile([1, ns], fp32)
                nc.gpsimd.memset(ones16[:], 1.0)
                b_row = wpool.tile([1, D], fp32)
                nc.sync.dma_start(b_row[:], b_d.ap()[:])
                w_sb = wpool.tile([128, JC, S_IN], fp32)
                nc.sync.dma_start(
                    w_sb[:], w_d.ap().rearrange("(dc p) s -> p dc s", p=128))
                wt = wpool.tile([128, SC, D], fp32)
                for dc in range(JC):
                    for sc in range(SC):
                        pst = ps_b.tile([128, 512], fp32, tag="bldps")
                        nc.tensor.transpose(
                            pst[:, :128],
                            w_sb[:, dc, sc * 128:(sc + 1) * 128], identf[:])
                        nc.vector.tensor_copy(
                            wt[:, sc, dc * 128:(dc + 1) * 128], pst[:, :128])
                for h in range(2):
                    ps_src = ps_b.tile([128, 512], fp32, tag="bldps")
                    for sc in range(SC):
                        nc.tensor.matmul(
                            ps_src[:ns], xt[:, sc, :],
                            wt[:, sc, h * 512:(h + 1) * 512],
                            start=(sc == 0), stop=False)
                    nc.tensor.matmul(
                        ps_src[:ns], ones16[:],
                        b_row[:, h * 512:(h + 1) * 512],
                        start=False, stop=True)
                    nc.scalar.activation(
                        src_sb[:, h * 512:(h + 1) * 512], ps_src[:ns],
                        AF.Copy)
                for c in range(JC):
                    pst = ps_b.tile([128, 512], fp32, tag="bldps")
                    nc.tensor.transpose(
                        pst[:, :ns], src_sb[:, c * 128:(c + 1) * 128],
                        identf[:ns, :ns])
                    nc.vector.tensor_copy(srccol[:, :, c], pst[:, :ns])

            pacc = apool.tile([128, JC, D], fp32)
            nc.gpsimd.memset(pacc[:], 0.0)

            # ------------- phase 2: per-sample Sinkhorn ---------------------
            with (
                tc.tile_pool(name="mats", bufs=group) as mpool,
                tc.tile_pool(name="rows", bufs=1) as wrow,
                tc.tile_pool(name="vecs", bufs=1) as vpool,
                tc.tile_pool(name="ptl", bufs=1) as ppool,
            ):
                for g0 in range(0, ns, group):
                    gidx = list(range(g0, g0 + group))
                    st = {}
                    for i in gidx:
                        gslot = i % group
                        base = 32 * gslot
                        M = mpool.tile([128, JC, D], bf16, tag="M")
                        MT = mpool.tile([128, JC, D], bf16, tag="MT")
                        svf = vpool.tile([128, 64], fp32, tag=f"svf{gslot}")
                        svb = vpool.tile([128, 16], bf16, tag=f"svb{gslot}")
                        svh = vpool.tile([128, 16], fp16, tag=f"svh{gslot}")
                        urow = vpool.tile([1, D], fp32, tag=f"ur{gslot}")
                        vrow = vpool.tile([1, D], fp32, tag=f"vr{gslot}")
                        yh = wrow.tile([1, D], fp16, tag=f"yh{gslot}")
                        yl = wrow.tile([1, D], fp16, tag=f"yl{gslot}")
                        sh = wrow.tile([1, D], fp16, tag=f"sh{gslot}")
                        sl = wrow.tile([1, D], fp16, tag=f"sl{gslot}")
                        ah = wrow.tile([1, D], fp16, tag=f"ah{gslot}")
                        al = wrow.tile([1, D], fp16, tag=f"al{gslot}")
                        uscr = dpool.tile([D], fp32, tag=f"uscr{gslot}")
                        vscr = dpool.tile([D], fp32, tag=f"vscr{gslot}")
                        hscr = dpool.tile([D], fp16, tag=f"hscr{gslot}")
                        lscr = dpool.tile([D], fp16, tag=f"lscr{gslot}")

                        # --- fp16 splits of sqrt(2g)*Y and sqrt(2g)*src ---
                        # (urow, vrow used as f32 scratch rows here)
                        for (row_src, hi, lo) in ((y_sb, yh, yl),
                                                  (src_sb, sh, sl)):
                            nc.sync.dma_start(urow[:], row_src[i:i + 1, :])
                            nc.vector.tensor_scalar_mul(urow[:], urow[:],
                                                        RT2G)
                            nc.vector.tensor_copy(hi[:], urow[:])
                            nc.vector.tensor_copy(vrow[:], hi[:])
                            nc.vector.tensor_sub(urow[:], urow[:], vrow[:])
                            nc.vector.tensor_copy(lo[:], urow[:])
                        # s2 col = gamma*src^2
                        nc.scalar.activation(svf[:, S2C], srccol[:, i, :],
                                             AF.Square, scale=RTG)
                        nc.gpsimd.memset(svf[:, BCC], 0.0)
                        st[i] = dict(M=M, MT=MT, svf=svf, svb=svb, svh=svh,
                                     urow=urow, vrow=vrow, yh=yh, yl=yl,
                                     sh=sh, sl=sl, ah=ah, al=al, uscr=uscr,
                                     vscr=vscr, hscr=hscr, lscr=lscr,
                                     base=base)

                    def rowterm_split(d):
                        """aux = s2 - BC in col space; fp16 split; to rows."""
                        svf, svh = d["svf"], d["svh"]
                        nc.vector.tensor_sub(svf[:, AUXC], svf[:, S2C],
                                             svf[:, BCC])
                        nc.vector.tensor_copy(svh[:, 0:8], svf[:, AUXC])
                        nc.vector.tensor_copy(svf[:, LNX], svh[:, 0:8])
                        nc.vector.tensor_sub(svf[:, LNX], svf[:, AUXC],
                                             svf[:, LNX])
                        nc.vector.tensor_copy(svh[:, 8:16], svf[:, LNX])
                        nc.sync.dma_start(
                            d["hscr"][:].rearrange("(c p) -> p c", p=128),
                            svh[:, 0:8])
                        nc.sync.dma_start(d["ah"][:], d["hscr"][:])
                        nc.sync.dma_start(
                            d["lscr"][:].rearrange("(c p) -> p c", p=128),
                            svh[:, 8:16])
                        nc.sync.dma_start(d["al"][:], d["lscr"][:])

                    def pre_psum(d, jc, h, ps):
                        """ps = cross - ones*(s2 - BC)  via fp16-split rank-1s"""
                        ja, jb = jc * 128, (jc + 1) * 128
                        ha, hb = h * 512, (h + 1) * 512
                        nc.tensor.matmul(ps[:], d["yh"][:, ja:jb],
                                         d["sh"][:, ha:hb],
                                         start=True, stop=False)
                        nc.tensor.matmul(ps[:], d["yh"][:, ja:jb],
                                         d["sl"][:, ha:hb],
                                         start=False, stop=False)
                        nc.tensor.matmul(ps[:], d["yl"][:, ja:jb],
                                         d["sh"][:, ha:hb],
                                         start=False, stop=False)
                        nc.tensor.matmul(ps[:], neg1h[:], d["ah"][:, ha:hb],
                                         start=False, stop=False)
                        nc.tensor.matmul(ps[:], neg1h[:], d["al"][:, ha:hb],
                                         start=False, stop=True)

                    # ---- init pass: BRC = -max_k(pre with BC=0) ----
                    for i in gidx:
                        d = st[i]
                        svf = d["svf"]
                        rowterm_split(d)
                        for jc in range(JC):
                            for h in range(2):
                                ps = ps_b.tile([128, 512], fp32, tag="bldps")
                                pre_psum(d, jc, h, ps)
                                nc.vector.tensor_reduce(
                                    out=svf[:, RM][:, h:h + 1], in_=ps[:],
                                    op=ALU.max, axis=AX.X)
                            nc.vector.tensor_max(
                                svf[:, RM][:, 0:1], svf[:, RM][:, 0:1],
                                svf[:, RM][:, 1:2])
                            nc.vector.tensor_scalar_mul(
                                svf[:, BRC][:, jc:jc + 1],
                                svf[:, RM][:, 0:1], -1.0)

                    # ---- segment loop: rebuild + seg_len iterations --------
                    def seg_body():
                        for i in gidx:
                            d = st[i]
                            svf, svb = d["svf"], d["svb"]
                            rowterm_split(d)
                            for jc in range(JC):
                                for h in range(2):
                                    ps = ps_b.tile([128, 512], fp32,
                                                   tag="bldps")
                                    pre_psum(d, jc, h, ps)
                                    nc.scalar.activation(
                                        d["M"][:, jc, h * 512:(h + 1) * 512],
                                        ps[:], AF.Exp,
                                        bias=svf[:, BRC][:, jc:jc + 1])
                                # MT chunks of this jc via DMA transpose
                                for kc in range(JC):
                                    nc.sync.dma_start(
                                        d["MT"][:, kc,
                                                jc * 128:(jc + 1) * 128],
                                        d["M"][:, jc,
                                               kc * 128:(kc + 1) * 128],
                                        transpose=True)
                            nc.gpsimd.memset(svb[:, 8:16], 1.0)  # vcol = 1
                        for t in range(seg_len):
                            for i in gidx:
                                d = st[i]
                                svf, svb, base = d["svf"], d["svb"], d["base"]
                                vcol = svb[:, 8:16]
                                ucol = svb[:, 0:8]
                                pss = ps_i.tile([128, D], fp32, tag="itps")
                                for h in range(2):
                                    for kc in range(JC):
                                        nc.tensor.matmul(
                                            pss[base:base + 1,
                                                h * 512:(h + 1) * 512],
                                            vcol[:, kc:kc + 1],
                                            d["MT"][:, kc,
                                                    h * 512:(h + 1) * 512],
                                            start=(kc == 0),
                                            stop=(kc == JC - 1),
                                            tile_position=(0, base))
                                nc.vector.reciprocal(
                                    d["urow"][:], pss[base:base + 1, :])
                                nc.sync.dma_start(d["uscr"][:], d["urow"][:])
                                nc.sync.dma_start(
                                    svf[:, UCF],
                                    d["uscr"][:].rearrange("(c p) -> p c",
                                                           p=128))
                                nc.vector.tensor_copy(ucol[:], svf[:, UCF])
                            for i in gidx:
                                d = st[i]
                                svf, svb, base = d["svf"], d["svb"], d["base"]
                                ucol = svb[:, 0:8]
                                vcol = svb[:, 8:16]
                                pst_ = ps_i.tile([128, D], fp32, tag="itps")
                                for h in range(2):
                                    for jc in range(JC):
                                        nc.tensor.matmul(
                                            pst_[base:base + 1,
                                                 h * 512:(h + 1) * 512],
                                            ucol[:, jc:jc + 1],
                                            d["M"][:, jc,
                                                   h * 512:(h + 1) * 512],
                                            start=(jc == 0),
                                            stop=(jc == JC - 1),
                                            tile_position=(0, base))
                                nc.vector.reciprocal(
                                    d["vrow"][:], pst_[base:base + 1, :])
                                nc.sync.dma_start(d["vscr"][:], d["vrow"][:])
                                nc.sync.dma_start(
                                    svf[:, VCF],
                                    d["vscr"][:].rearrange("(c p) -> p c",
                                                           p=128))
                                nc.vector.tensor_copy(vcol[:], svf[:, VCF])
                        # absorb: BRC += ln u, BCC += ln v
                        for i in gidx:
                            svf = st[i]["svf"]
                            nc.scalar.activation(svf[:, LNX], svf[:, UCF],
                                                 AF.Ln)
                            nc.vector.tensor_add(svf[:, BRC], svf[:, BRC],
                                                 svf[:, LNX])
                            nc.scalar.activation(svf[:, LNX], svf[:, VCF],
                                                 AF.Ln)
                            nc.vector.tensor_add(svf[:, BCC], svf[:, BCC],
                                                 svf[:, LNX])

                    if n_seg > 1:
                        with tc.For_i(0, n_seg, 1, hint_engines=(ET.PE,)):
                            seg_body()
                    else:
                        seg_body()

                    # ---- P accumulation: D*P = exp(pre + BRC) --------------
                    for i in gidx:
                        d = st[i]
                        svf = d["svf"]
                        rowterm_split(d)
                        for jc in range(JC):
                            for h in range(2):
                                ps = ps_b.tile([128, 512], fp32, tag="bldps")
                                pre_psum(d, jc, h, ps)
                                ptile = ppool.tile([128, 512], fp32,
                                                   tag="ptile")
                                nc.scalar.activation(
                                    ptile[:], ps[:], AF.Exp,
                                    bias=svf[:, BRC][:, jc:jc + 1])
                                nc.vector.tensor_add(
                                    pacc[:, jc, h * 512:(h + 1) * 512],
                                    pacc[:, jc, h * 512:(h + 1) * 512],
                                    ptile[:])

            # ------------- phase 3: AllReduce + finale ----------------------
            pacc_b = dpool.tile([D, D], fp32)
            pall_b = dpool.tile(
                [D, D], fp32,
                addr_space="Shared" if n_cores > 4 else "Local")
            nc.sync.dma_start(
                pacc_b[:].rearrange("(jc p) k -> p jc k", p=128), pacc[:])
            if skip_collective:
                nc.sync.dma_start(pall_b[:], pacc_b[:])
            else:
                nc.gpsimd.collective_compute(
                    "AllReduce", ALU.add,
                    replica_groups=[list(range(n_cores))],
                    ins=[pacc_b.opt()], outs=[pall_b.opt()],
                )
            with tc.tile_pool(name="fin", bufs=1) as fpool:
                ot = fpool.tile([128, JC, D], fp32)
                nc.sync.dma_start(
                    ot[:], pall_b[:].rearrange("(jc p) k -> p jc k", p=128))
                dl = fpool.tile([128, JC, D], fp32)
                nc.sync.dma_start(
                    dl[:],
                    delta_d.ap().rearrange("(jc p) k -> p jc k", p=128))
                nc.vector.tensor_scalar_mul(ot[:], ot[:], SCALE / N_GLOB)
                nc.vector.tensor_add(ot[:], ot[:], dl[:])
                out_sb = fpool.tile([ns, D], fp32)
                for h in range(2):
                    pso = ps_b.tile([128, 512], fp32, tag="bldps")
                    for jc in range(JC):
                        nc.tensor.matmul(
                            pso[:ns], srccol[:, :, jc],
                            ot[:, jc, h * 512:(h + 1) * 512],
                            start=(jc == 0), stop=(jc == JC - 1))
                    nc.scalar.activation(
                        out_sb[:, h * 512:(h + 1) * 512], pso[:ns], AF.Copy)
                nc.sync.dma_start(out_d.ap()[:], out_sb[:])

    nc.compile()
    return nc


def kernel(**inputs):
    X = np.ascontiguousarray(inputs["X"], np.float32)
    Y = np.ascontiguousarray(inputs["Y"], np.float32)
    W = np.ascontiguousarray(inputs["W"], np.float32)
    b = np.ascontiguousarray(inputs["b"], np.float32).reshape(1, D)
    delta = np.ascontiguousarray(inputs["delta_ot"], np.float32)

    from concourse import bass_utils

    if "nc" not in _cache:
        _cache["nc"] = build()
    nc = _cache["nc"]

    in_maps = []
    for c in range(N_CORES):
        sl = slice(c * NS, (c + 1) * NS)
        in_maps.append({
            "x": X[sl], "y": Y[sl], "w": W, "bvec": b, "delta": delta,
        })
    res = bass_utils.run_bass_kernel_spmd(
        nc, in_maps, core_ids=list(range(N_CORES)))
    out = np.concatenate([res.results[c]["out"] for c in range(N_CORES)],
                         axis=0)
    return out.astype(np.float32)


if __name__ == "__main__":
    import reference
    ins = reference.setup_inputs()
    ins = {k: np.asarray(v) for k, v in ins.items()}
    got = kernel(**ins)
    print("out", got.shape, got.dtype)


# revision 25
# speedup vs baseline: 1.6061x; 1.6061x over previous
"""Trainium2 Bass kernel for nn_AlignerOT: per-sample entropic Sinkhorn OT aligner.

reference math:
    src = X @ W.T + b                                   [N, D]
    cost[i,j,k] = (src[i,k] - Y[i,j])^2 * SCALE         [N, D, D]
    P[i] = sinkhorn_log(cost[i], uniform, uniform)      50 iters, eps=0.1
    ot = P.mean(0) * D * SCALE + delta_ot
    out = src @ ot

Device algorithm (math-equivalent, per sample i):
    gamma = SCALE/eps = 3000; cross_jk = 2*gamma*Y_j*src_k; s2_k = gamma*src_k^2
    pre_jk = cross_jk - (s2_k - BC_k)   built in PSUM by a K=3-packed fp16
             matmul (yh,yh,yl)x(sh,sl,sh) plus two rank-1 rowterm matmuls
             (fp16 hi/lo splits reproduce the f32 values; validated 7e-4)
    The log-domain Sinkhorn trajectory is reproduced exactly by kernel-space
    iterations u <- 1/(M v), v <- 1/(M^T u) on M = exp(pre + BR_j),
    restabilized every R=10 iterations by absorbing ln u, ln v into row/col
    biases BR, BC and rebuilding M (the dual potentials span thousands of
    decades, so linear-domain vectors overflow f32 past ~20 iterations;
    segment chaining is exact).  50 iters = 5 segments x 10.
    MT = transpose(M) via PE transpose (exact).  Row<->column vector
    relayouts use rank-1 "transpose trick" matmuls (no DMA on the critical
    path).  The two group samples' matvec streams run concurrently on PE
    col-groups 0 and 32 (measured 2.0x).
    P[i]*D = exp(pre + BR) after the final absorb.
Sharding: data-parallel over N (16 samples/core); AllReduce sum_i P[i]*D;
    ot = (SCALE/N)*AR + delta; out rows per-core; host concat.
"""

import numpy as np

N_CORES = 8
N_GLOB = 128
NS = N_GLOB // N_CORES   # 16
S_IN = 768
SC = S_IN // 128         # 6
D = 1024
JC = D // 128            # 8
EPS = 0.1
SCALE = 300.0
GAMMA = SCALE / EPS
RT2G = float(np.sqrt(2.0 * GAMMA))
RTG = float(np.sqrt(GAMMA))
N_ITERS = 50
N_SEG = 5
GROUP = 2

_cache = {}


def build(n_iters=N_ITERS, n_seg=N_SEG, ns=NS, group=GROUP, n_cores=N_CORES,
          skip_collective=False):
    import concourse.bass as bass
    import concourse.bacc as bacc
    import concourse.tile as tile
    import concourse.mybir as mybir
    from concourse.masks import make_identity

    fp32 = mybir.dt.float32
    bf16 = mybir.dt.bfloat16
    fp16 = mybir.dt.float16
    AF = mybir.ActivationFunctionType
    ALU = mybir.AluOpType
    AX = mybir.AxisListType
    ET = mybir.EngineType

    nc = bacc.Bacc("TRN2", target_bir_lowering=False, debug=False,
                   num_devices=n_cores)

    x_d = nc.dram_tensor("x", [ns, S_IN], fp32, kind="ExternalInput")
    y_d = nc.dram_tensor("y", [ns, D], fp32, kind="ExternalInput")
    w_d = nc.dram_tensor("w", [D, S_IN], fp32, kind="ExternalInput")
    b_d = nc.dram_tensor("bvec", [1, D], fp32, kind="ExternalInput")
    delta_d = nc.dram_tensor("delta", [D, D], fp32, kind="ExternalInput")
    out_d = nc.dram_tensor("out", [ns, D], fp32, kind="ExternalOutput")

    assert ns % group == 0
    assert n_iters % n_seg == 0
    seg_len = n_iters // n_seg

    # packed per-group-slot column vectors inside svf [128, 64] f32:
    UCF, VCF, BRC, BCC, LNX, S2C, AUXC = (
        slice(0, 8), slice(8, 16), slice(16, 24), slice(24, 32),
        slice(32, 40), slice(40, 48), slice(48, 56))
    RM = slice(56, 58)

    with tile.TileContext(nc) as tc:
        with (
            tc.tile_pool(name="const", bufs=1) as cpool,
            tc.tile_pool(name="rdata", bufs=1) as rpool,
            tc.tile_pool(name="acc", bufs=1) as apool,
            tc.tile_pool(name="ps_b", bufs=2, space="PSUM") as ps_b,
            tc.tile_pool(name="ps_i", bufs=2, space="PSUM") as ps_i,
            tc.tile_pool(name="ps_s", bufs=2, space="PSUM") as ps_s,
            tc.tile_pool(name="dram", bufs=2, space="DRAM") as dpool,
        ):
            identb = cpool.tile([128, 128], bf16)
            make_identity(nc, identb[:])
            identh = cpool.tile([128, 128], fp16)
            make_identity(nc, identh[:])
            oneb = cpool.tile([1, 1], bf16)
            nc.gpsimd.memset(oneb[:], 1.0)
            neg1h = cpool.tile([1, 128], fp16)
            nc.gpsimd.memset(neg1h[:], -1.0)

            # ---------------- phase 1: src = X @ W.T + b --------------------
            src_sb = rpool.tile([ns, D], fp32)
            y_sb = rpool.tile([ns, D], fp32)
            nc.sync.dma_start(y_sb[:], y_d.ap()[:])
            srccol = rpool.tile([128, ns, JC], fp32)
            with tc.tile_pool(name="wls", bufs=1) as wpool:
                identf = wpool.tile([128, 128], fp32)
                make_identity(nc, identf[:])
                xt = wpool.tile([128, SC, ns], fp32)
                for sc in range(SC):
                    nc.sync.dma_start(
                        xt[:, sc, :],
                        x_d.ap()[:, sc * 128:(sc + 1) * 128].rearrange(
                            "n p -> p n"))
                ones16 = wpool.tile([1, ns], fp32)
                nc.gpsimd.memset(ones16[:], 1.0)
                b_row = wpool.tile([1, D], fp32)
                nc.sync.dma_start(b_row[:], b_d.ap()[:])
                w_sb = wpool.tile([128, JC, S_IN], fp32)
                nc.sync.dma_start(
                    w_sb[:], w_d.ap().rearrange("(dc p) s -> p dc s", p=128))
                wt = wpool.tile([128, SC, D], fp32)
                for dc in range(JC):
                    for sc in range(SC):
                        pst = ps_b.tile([128, 512], fp32, tag="bldps")
                        nc.tensor.transpose(
                            pst[:, :128],
                            w_sb[:, dc, sc * 128:(sc + 1) * 128], identf[:])
                        nc.vector.tensor_copy(
                            wt[:, sc, dc * 128:(dc + 1) * 128], pst[:, :128])
                for h in range(2):
                    ps_src = ps_b.tile([128, 512], fp32, tag="bldps")
                    for sc in range(SC):
                        nc.tensor.matmul(
                            ps_src[:ns], xt[:, sc, :],
                            wt[:, sc, h * 512:(h + 1) * 512],
                            start=(sc == 0), stop=False)
                    nc.tensor.matmul(
                        ps_src[:ns], ones16[:],
                        b_row[:, h * 512:(h + 1) * 512],
                        start=False, stop=True)
                    nc.scalar.activation(
                        src_sb[:, h * 512:(h + 1) * 512], ps_src[:ns],
                        AF.Copy)
                for c in range(JC):
                    pst = ps_b.tile([128, 512], fp32, tag="bldps")
                    nc.tensor.transpose(
                        pst[:, :ns], src_sb[:, c * 128:(c + 1) * 128],
                        identf[:ns, :ns])
                    nc.vector.tensor_copy(srccol[:, :, c], pst[:, :ns])

            pacc = apool.tile([128, JC, D], fp32)
            nc.gpsimd.memset(pacc[:], 0.0)

            # ------------- phase 2: per-sample Sinkhorn ---------------------
            # lb rows: (yh, yh, yl); rb rows: (sh, sl, sh)  [fp16, K=3 pack]
            # rowterm rows rth/rtl are separate partition-0 [1,D] fp16 tiles
            with (
                tc.tile_pool(name="mats", bufs=group) as mpool,
                tc.tile_pool(name="rows", bufs=1) as wrow,
                tc.tile_pool(name="vecs", bufs=1) as vpool,
                tc.tile_pool(name="ptl", bufs=1) as ppool,
            ):
                for g0 in range(0, ns, group):
                    gidx = list(range(g0, g0 + group))
                    st = {}
                    for i in gidx:
                        gslot = i % group
                        base = 32 * gslot
                        M = mpool.tile([128, JC, D], bf16, tag="M")
                        MT = mpool.tile([128, JC, D], bf16, tag="MT")
                        svf = vpool.tile([128, 64], fp32, tag=f"svf{gslot}")
                        svb = vpool.tile([128, 16], bf16, tag=f"svb{gslot}")
                        svh = vpool.tile([128, 16], fp16, tag=f"svh{gslot}")
                        srow = vpool.tile([1, D], bf16, tag=f"sr{gslot}")
                        lb = wrow.tile([3, D], fp16, tag=f"lb{gslot}")
                        rb = wrow.tile([3, D], fp16, tag=f"rb{gslot}")
                        rth = wrow.tile([1, D], fp16, tag=f"rth{gslot}")
                        rtl = wrow.tile([1, D], fp16, tag=f"rtl{gslot}")
                        scr = wrow.tile([1, D], fp32, tag="scr")
                        sc2 = wrow.tile([1, D], fp32, tag="sc2")
                        s16 = wrow.tile([1, D], fp16, tag="s16")

                        # fp16 hi/lo splits of sqrt(2g)*Y (lbank) and
                        # sqrt(2g)*src (rbank)
                        for (srcrow, bank, hi_rows, lo_row) in (
                                (y_sb, lb, (0, 1), 2),
                                (src_sb, rb, (0, 2), 1)):
                            nc.sync.dma_start(scr[:], srcrow[i:i + 1, :])
                            nc.vector.tensor_scalar_mul(scr[:], scr[:],
                                                        RT2G)
                            nc.vector.tensor_copy(s16[:], scr[:])   # hi
                            for r in hi_rows:
                                nc.sync.dma_start(bank[r:r + 1, :], s16[:])
                            nc.vector.tensor_copy(sc2[:], s16[:])
                            nc.vector.tensor_sub(scr[:], scr[:], sc2[:])
                            nc.vector.tensor_copy(s16[:], scr[:])   # lo
                            nc.sync.dma_start(bank[lo_row:lo_row + 1, :],
                                              s16[:])
                        # s2 col = gamma*src^2
                        nc.scalar.activation(svf[:, S2C], srccol[:, i, :],
                                             AF.Square, scale=RTG)
                        nc.gpsimd.memset(svf[:, BCC], 0.0)
                        st[i] = dict(M=M, MT=MT, svf=svf, svb=svb, svh=svh,
                                     srow=srow, lb=lb, rb=rb, rth=rth,
                                     rtl=rtl, base=base)

                    def rowterm_update(d):
                        """rowterm = s2 - BC in cols; fp16 split; transpose
                        into rbank rows 1 (hi) and 4 (lo) via rank-1 trick."""
                        svf, svh = d["svf"], d["svh"]
                        nc.vector.tensor_sub(svf[:, AUXC], svf[:, S2C],
                                             svf[:, BCC])
                        nc.vector.tensor_copy(svh[:, 0:8], svf[:, AUXC])
                        nc.vector.tensor_copy(svf[:, LNX], svh[:, 0:8])
                        nc.vector.tensor_sub(svf[:, LNX], svf[:, AUXC],
                                             svf[:, LNX])
                        nc.vector.tensor_copy(svh[:, 8:16], svf[:, LNX])
                        for (cols, dstrow) in ((svh[:, 0:8], d["rth"]),
                                               (svh[:, 8:16], d["rtl"])):
                            for h in range(2):
                                psr = ps_b.tile([128, 512], fp32,
                                                tag="bldps")
                                for c in range(4):
                                    cc = h * 4 + c
                                    nc.tensor.matmul(
                                        psr[0:1, c * 128:(c + 1) * 128],
                                        cols[:, cc:cc + 1], identh[:],
                                        start=True, stop=True)
                                nc.scalar.activation(
                                    dstrow[0:1, h * 512:(h + 1) * 512],
                                    psr[0:1, :], AF.Copy)

                    def pre_psum(d, jc, h, ps):
                        """ps = cross - (s2 - BC) via 2 K-packed fp16 MMs."""
                        ja, jb = jc * 128, (jc + 1) * 128
                        ha, hb = h * 512, (h + 1) * 512
                        nc.tensor.matmul(ps[:], d["lb"][0:3, ja:jb],
                                         d["rb"][0:3, ha:hb],
                                         start=True, stop=False)
                        nc.tensor.matmul(ps[:], neg1h[:],
                                         d["rth"][:, ha:hb],
                                         start=False, stop=False)
                        nc.tensor.matmul(ps[:], neg1h[:],
                                         d["rtl"][:, ha:hb],
                                         start=False, stop=True)

                    # ---- init pass: BRC = -max_k(pre with BC=0) ----
                    for i in gidx:
                        d = st[i]
                        svf = d["svf"]
                        rowterm_update(d)
                        for jc in range(JC):
                            for h in range(2):
                                ps = ps_b.tile([128, 512], fp32, tag="bldps")
                                pre_psum(d, jc, h, ps)
                                nc.vector.tensor_reduce(
                                    out=svf[:, RM][:, h:h + 1], in_=ps[:],
                                    op=ALU.max, axis=AX.X)
                            nc.vector.tensor_max(
                                svf[:, RM][:, 0:1], svf[:, RM][:, 0:1],
                                svf[:, RM][:, 1:2])
                            nc.vector.tensor_scalar_mul(
                                svf[:, BRC][:, jc:jc + 1],
                                svf[:, RM][:, 0:1], -1.0)

                    # ---- segment loop --------------------------------------
                    def side(d, mat, out_cols, out_colsb, use_act):
                        """one matvec side: out = 1/(mat @ invec) in columns"""
                        base = d["base"]
                        pss = ps_i.tile([128, D], fp32, tag="ivps")
                        for h in range(2):
                            for kc in range(JC):
                                nc.tensor.matmul(
                                    pss[base:base + 1,
                                        h * 512:(h + 1) * 512],
                                    d["invec"][:, kc:kc + 1],
                                    mat[:, kc, h * 512:(h + 1) * 512],
                                    start=(kc == 0), stop=(kc == JC - 1),
                                    tile_position=(0, base))
                        if use_act:
                            nc.scalar.activation(d["srow"][:],
                                                 pss[base:base + 1, :],
                                                 AF.Copy)
                        else:
                            nc.vector.tensor_copy(d["srow"][:],
                                                  pss[base:base + 1, :])
                        psc = ps_s.tile([128, 128], fp32, tag="smallps")
                        for c in range(JC):
                            nc.tensor.matmul(
                                psc[:, c:c + 1],
                                d["srow"][0:1, c * 128:(c + 1) * 128],
                                oneb[:], start=True, stop=True)
                        nc.vector.reciprocal(out_cols[:], psc[:, :JC])
                        nc.vector.tensor_copy(out_colsb[:], out_cols[:])

                    def seg_body():
                        for i in gidx:
                            d = st[i]
                            svf, svb = d["svf"], d["svb"]
                            rowterm_update(d)
                            for jc in range(JC):
                                for h in range(2):
                                    ps = ps_b.tile([128, 512], fp32,
                                                   tag="bldps")
                                    pre_psum(d, jc, h, ps)
                                    nc.scalar.activation(
                                        d["M"][:, jc, h * 512:(h + 1) * 512],
                                        ps[:], AF.Exp,
                                        bias=svf[:, BRC][:, jc:jc + 1])
                                for kc in range(JC):
                                    pst = ps_s.tile([128, 128], bf16,
                                                    tag="smallps")
                                    nc.tensor.transpose(
                                        pst[:],
                                        d["M"][:, jc,
                                               kc * 128:(kc + 1) * 128],
                                        identb[:])
                                    nc.vector.tensor_copy(
                                        d["MT"][:, kc,
                                                jc * 128:(jc + 1) * 128],
                                        pst[:])
                            nc.gpsimd.memset(svb[:, 8:16], 1.0)  # vcol = 1
                        for t in range(seg_len):
                            for i in gidx:
                                d = st[i]
                                d["invec"] = d["svb"][:, 8:16]   # vcol
                                side(d, d["MT"], d["svf"][:, UCF],
                                     d["svb"][:, 0:8], use_act=True)
                            for i in gidx:
                                d = st[i]
                                d["invec"] = d["svb"][:, 0:8]    # ucol
                                side(d, d["M"], d["svf"][:, VCF],
                                     d["svb"][:, 8:16], use_act=False)
                        for i in gidx:
                            svf = st[i]["svf"]
                            nc.scalar.activation(svf[:, LNX], svf[:, UCF],
                                                 AF.Ln)
                            nc.vector.tensor_add(svf[:, BRC], svf[:, BRC],
                                                 svf[:, LNX])
                            nc.scalar.activation(svf[:, LNX], svf[:, VCF],
                                                 AF.Ln)
                            nc.vector.tensor_add(svf[:, BCC], svf[:, BCC],
                                                 svf[:, LNX])

                    if n_seg > 1:
                        with tc.For_i(0, n_seg, 1, hint_engines=(ET.PE,)):
                            seg_body()
                    else:
                        seg_body()

                    # ---- P accumulation: D*P = exp(pre + BRC) --------------
                    for i in gidx:
                        d = st[i]
                        svf = d["svf"]
                        rowterm_update(d)
                        for jc in range(JC):
                            for h in range(2):
                                ps = ps_b.tile([128, 512], fp32, tag="bldps")
                                pre_psum(d, jc, h, ps)
                                ptile = ppool.tile([128, 512], fp32,
                                                   tag="ptile")
                                nc.scalar.activation(
                                    ptile[:], ps[:], AF.Exp,
                                    bias=svf[:, BRC][:, jc:jc + 1])
                                nc.vector.tensor_add(
                                    pacc[:, jc, h * 512:(h + 1) * 512],
                                    pacc[:, jc, h * 512:(h + 1) * 512],
                                    ptile[:])

            # ------------- phase 3: AllReduce + finale ----------------------
            pacc_b = dpool.tile([D, D], fp32)
            pall_b = dpool.tile(
                [D, D], fp32,
                addr_space="Shared" if n_cores > 4 else "Local")
            nc.sync.dma_start(
                pacc_b[:].rearrange("(jc p) k -> p jc k", p=128), pacc[:])
            if skip_collective:
                nc.sync.dma_start(pall_b[:], pacc_b[:])
            else:
                nc.gpsimd.collective_compute(
                    "AllReduce", ALU.add,
                    replica_groups=[list(range(n_cores))],
                    ins=[pacc_b.opt()], outs=[pall_b.opt()],
                )
            with tc.tile_pool(name="fin", bufs=1) as fpool:
                ot = fpool.tile([128, JC, D], fp32)
                nc.sync.dma_start(
                    ot[:], pall_b[:].rearrange("(jc p) k -> p jc k", p=128))
                dl = fpool.tile([128, JC, D], fp32)
                nc.sync.dma_start(
                    dl[:],
                    delta_d.ap().rearrange("(jc p) k -> p jc k", p=128))
                nc.vector.tensor_scalar_mul(ot[:], ot[:], SCALE / N_GLOB)
                nc.vector.tensor_add(ot[:], ot[:], dl[:])
                out_sb = fpool.tile([ns, D], fp32)
                for h in range(2):
                    pso = ps_b.tile([128, 512], fp32, tag="bldps")
                    for jc in range(JC):
                        nc.tensor.matmul(
                            pso[:ns], srccol[:, :, jc],
                            ot[:, jc, h * 512:(h + 1) * 512],
                            start=(jc == 0), stop=(jc == JC - 1))
                    nc.scalar.activation(
                        out_sb[:, h * 512:(h + 1) * 512], pso[:ns], AF.Copy)
                nc.sync.dma_start(out_d.ap()[:], out_sb[:])

    nc.compile()
    return nc


def kernel(**inputs):
    X = np.ascontiguousarray(inputs["X"], np.float32)
    Y = np.ascontiguousarray(inputs["Y"], np.float32)
    W = np.ascontiguousarray(inputs["W"], np.float32)
    b = np.ascontiguousarray(inputs["b"], np.float32).reshape(1, D)
    delta = np.ascontiguousarray(inputs["delta_ot"], np.float32)

    from concourse import bass_utils

    if "nc" not in _cache:
        _cache["nc"] = build()
    nc = _cache["nc"]

    in_maps = []
    for c in range(N_CORES):
        sl = slice(c * NS, (c + 1) * NS)
        in_maps.append({
            "x": X[sl], "y": Y[sl], "w": W, "bvec": b, "delta": delta,
        })
    res = bass_utils.run_bass_kernel_spmd(
        nc, in_maps, core_ids=list(range(N_CORES)))
    out = np.concatenate([res.results[c]["out"] for c in range(N_CORES)],
                         axis=0)
    return out.astype(np.float32)


if __name__ == "__main__":
    import reference
    ins = reference.setup_inputs()
    ins = {k: np.asarray(v) for k, v in ins.items()}
    got = kernel(**ins)
    print("out", got.shape, got.dtype)


# revision 26
# speedup vs baseline: 1.6803x; 1.0462x over previous
"""Trainium2 Bass kernel for nn_AlignerOT: per-sample entropic Sinkhorn OT aligner.

reference math:
    src = X @ W.T + b                                   [N, D]
    cost[i,j,k] = (src[i,k] - Y[i,j])^2 * SCALE         [N, D, D]
    P[i] = sinkhorn_log(cost[i], uniform, uniform)      50 iters, eps=0.1
    ot = P.mean(0) * D * SCALE + delta_ot
    out = src @ ot

Device algorithm (math-equivalent, per sample i):
    gamma = SCALE/eps = 3000; cross_jk = 2*gamma*Y_j*src_k; s2_k = gamma*src_k^2
    pre_jk = cross_jk - (s2_k - BC_k)   built in PSUM by a K=3-packed fp16
             matmul (yh,yh,yl)x(sh,sl,sh) plus two rank-1 rowterm matmuls
             (fp16 hi/lo splits reproduce the f32 values; validated 7e-4)
    The log-domain Sinkhorn trajectory is reproduced exactly by kernel-space
    iterations u <- 1/(M v), v <- 1/(M^T u) on M = exp(pre + BR_j),
    restabilized every R=10 iterations by absorbing ln u, ln v into row/col
    biases BR, BC and rebuilding M (the dual potentials span thousands of
    decades, so linear-domain vectors overflow f32 past ~20 iterations;
    segment chaining is exact).  50 iters = 5 segments x 10.
    MT = transpose(M) via PE transpose (exact).  Row<->column vector
    relayouts use rank-1 "transpose trick" matmuls (no DMA on the critical
    path).  The two group samples' matvec streams run concurrently on PE
    col-groups 0 and 32 (measured 2.0x).
    P[i]*D = exp(pre + BR) after the final absorb.
Sharding: data-parallel over N (16 samples/core); AllReduce sum_i P[i]*D;
    ot = (SCALE/N)*AR + delta; out rows per-core; host concat.
"""

import numpy as np

N_CORES = 8
N_GLOB = 128
NS = N_GLOB // N_CORES   # 16
S_IN = 768
SC = S_IN // 128         # 6
D = 1024
JC = D // 128            # 8
EPS = 0.1
SCALE = 300.0
GAMMA = SCALE / EPS
RT2G = float(np.sqrt(2.0 * GAMMA))
RTG = float(np.sqrt(GAMMA))
N_ITERS = 50
N_SEG = 5
GROUP = 2

_cache = {}


def build(n_iters=N_ITERS, n_seg=N_SEG, ns=NS, group=GROUP, n_cores=N_CORES,
          skip_collective=False):
    import concourse.bass as bass
    import concourse.bacc as bacc
    import concourse.tile as tile
    import concourse.mybir as mybir
    from concourse.masks import make_identity

    fp32 = mybir.dt.float32
    bf16 = mybir.dt.bfloat16
    fp16 = mybir.dt.float16
    AF = mybir.ActivationFunctionType
    ALU = mybir.AluOpType
    AX = mybir.AxisListType
    ET = mybir.EngineType

    nc = bacc.Bacc("TRN2", target_bir_lowering=False, debug=False,
                   num_devices=n_cores)

    x_d = nc.dram_tensor("x", [ns, S_IN], fp32, kind="ExternalInput")
    y_d = nc.dram_tensor("y", [ns, D], fp32, kind="ExternalInput")
    w_d = nc.dram_tensor("w", [D, S_IN], fp32, kind="ExternalInput")
    b_d = nc.dram_tensor("bvec", [1, D], fp32, kind="ExternalInput")
    delta_d = nc.dram_tensor("delta", [D, D], fp32, kind="ExternalInput")
    out_d = nc.dram_tensor("out", [ns, D], fp32, kind="ExternalOutput")

    assert ns % group == 0
    assert n_iters % n_seg == 0
    seg_len = n_iters // n_seg

    # packed per-group-slot column vectors inside svf [128, 64] f32:
    UCF, VCF, BRC, BCC, LNX, S2C, AUXC = (
        slice(0, 8), slice(8, 16), slice(16, 24), slice(24, 32),
        slice(32, 40), slice(40, 48), slice(48, 56))
    RM = slice(56, 58)

    with tile.TileContext(nc) as tc:
        with (
            tc.tile_pool(name="const", bufs=1) as cpool,
            tc.tile_pool(name="rdata", bufs=1) as rpool,
            tc.tile_pool(name="acc", bufs=1) as apool,
            tc.tile_pool(name="ps_b", bufs=2, space="PSUM") as ps_b,
            tc.tile_pool(name="ps_i", bufs=2, space="PSUM") as ps_i,
            tc.tile_pool(name="ps_s", bufs=2, space="PSUM") as ps_s,
            tc.tile_pool(name="dram", bufs=2, space="DRAM") as dpool,
        ):
            identb = cpool.tile([128, 128], bf16)
            make_identity(nc, identb[:])
            identh = cpool.tile([128, 128], fp16)
            make_identity(nc, identh[:])
            oneb = cpool.tile([1, 1], bf16)
            nc.gpsimd.memset(oneb[:], 1.0)
            neg1h = cpool.tile([1, 128], fp16)
            nc.gpsimd.memset(neg1h[:], -1.0)

            # ---------------- phase 1: src = X @ W.T + b --------------------
            src_sb = rpool.tile([ns, D], fp32)
            y_sb = rpool.tile([ns, D], fp32)
            nc.sync.dma_start(y_sb[:], y_d.ap()[:])
            srccol = rpool.tile([128, ns, JC], fp32)
            with tc.tile_pool(name="wls", bufs=1) as wpool:
                identf = wpool.tile([128, 128], fp32)
                make_identity(nc, identf[:])
                xt = wpool.tile([128, SC, ns], fp32)
                for sc in range(SC):
                    nc.sync.dma_start(
                        xt[:, sc, :],
                        x_d.ap()[:, sc * 128:(sc + 1) * 128].rearrange(
                            "n p -> p n"))
                ones16 = wpool.tile([1, ns], fp32)
                nc.gpsimd.memset(ones16[:], 1.0)
                b_row = wpool.tile([1, D], fp32)
                nc.sync.dma_start(b_row[:], b_d.ap()[:])
                w_sb = wpool.tile([128, JC, S_IN], fp32)
                nc.sync.dma_start(
                    w_sb[:], w_d.ap().rearrange("(dc p) s -> p dc s", p=128))
                wt = wpool.tile([128, SC, D], fp32)
                for dc in range(JC):
                    for sc in range(SC):
                        pst = ps_b.tile([128, 512], fp32, tag="bldps")
                        nc.tensor.transpose(
                            pst[:, :128],
                            w_sb[:, dc, sc * 128:(sc + 1) * 128], identf[:])
                        nc.vector.tensor_copy(
                            wt[:, sc, dc * 128:(dc + 1) * 128], pst[:, :128])
                for h in range(2):
                    ps_src = ps_b.tile([128, 512], fp32, tag="bldps")
                    for sc in range(SC):
                        nc.tensor.matmul(
                            ps_src[:ns], xt[:, sc, :],
                            wt[:, sc, h * 512:(h + 1) * 512],
                            start=(sc == 0), stop=False)
                    nc.tensor.matmul(
                        ps_src[:ns], ones16[:],
                        b_row[:, h * 512:(h + 1) * 512],
                        start=False, stop=True)
                    nc.scalar.activation(
                        src_sb[:, h * 512:(h + 1) * 512], ps_src[:ns],
                        AF.Copy)
                for c in range(JC):
                    pst = ps_b.tile([128, 512], fp32, tag="bldps")
                    nc.tensor.transpose(
                        pst[:, :ns], src_sb[:, c * 128:(c + 1) * 128],
                        identf[:ns, :ns])
                    nc.vector.tensor_copy(srccol[:, :, c], pst[:, :ns])

            pacc = apool.tile([128, JC, D], fp32)
            nc.gpsimd.memset(pacc[:], 0.0)

            # ------------- phase 2: per-sample Sinkhorn ---------------------
            # lb rows: (yh, yh, yl); rb rows: (sh, sl, sh)  [fp16, K=3 pack]
            # rowterm rows rth/rtl are separate partition-0 [1,D] fp16 tiles
            with (
                tc.tile_pool(name="mats", bufs=group) as mpool,
                tc.tile_pool(name="rows", bufs=1) as wrow,
                tc.tile_pool(name="vecs", bufs=1) as vpool,
                tc.tile_pool(name="ptl", bufs=1) as ppool,
            ):
                for g0 in range(0, ns, group):
                    gidx = list(range(g0, g0 + group))
                    st = {}
                    for i in gidx:
                        gslot = i % group
                        base = 32 * gslot
                        M = mpool.tile([128, JC, D], bf16, tag="M")
                        MT = mpool.tile([128, JC, D], bf16, tag="MT")
                        svf = vpool.tile([128, 64], fp32, tag=f"svf{gslot}")
                        svb = vpool.tile([128, 16], bf16, tag=f"svb{gslot}")
                        svh = vpool.tile([128, 16], fp16, tag=f"svh{gslot}")
                        srow = vpool.tile([1, D], bf16, tag=f"sr{gslot}")
                        lb = wrow.tile([3, D], fp16, tag=f"lb{gslot}")
                        rb = wrow.tile([3, D], fp16, tag=f"rb{gslot}")
                        rth = wrow.tile([1, D], fp16, tag=f"rth{gslot}")
                        rtl = wrow.tile([1, D], fp16, tag=f"rtl{gslot}")
                        scr = wrow.tile([1, D], fp32, tag="scr")
                        sc2 = wrow.tile([1, D], fp32, tag="sc2")
                        s16 = wrow.tile([1, D], fp16, tag="s16")

                        # fp16 hi/lo splits of sqrt(2g)*Y (lbank) and
                        # sqrt(2g)*src (rbank)
                        for (srcrow, bank, hi_rows, lo_row) in (
                                (y_sb, lb, (0, 1), 2),
                                (src_sb, rb, (0, 2), 1)):
                            nc.sync.dma_start(scr[:], srcrow[i:i + 1, :])
                            nc.vector.tensor_scalar_mul(scr[:], scr[:],
                                                        RT2G)
                            nc.vector.tensor_copy(s16[:], scr[:])   # hi
                            for r in hi_rows:
                                nc.sync.dma_start(bank[r:r + 1, :], s16[:])
                            nc.vector.tensor_copy(sc2[:], s16[:])
                            nc.vector.tensor_sub(scr[:], scr[:], sc2[:])
                            nc.vector.tensor_copy(s16[:], scr[:])   # lo
                            nc.sync.dma_start(bank[lo_row:lo_row + 1, :],
                                              s16[:])
                        # s2 col = gamma*src^2
                        nc.scalar.activation(svf[:, S2C], srccol[:, i, :],
                                             AF.Square, scale=RTG)
                        nc.gpsimd.memset(svf[:, BCC], 0.0)
                        st[i] = dict(M=M, MT=MT, svf=svf, svb=svb, svh=svh,
                                     srow=srow, lb=lb, rb=rb, rth=rth,
                                     rtl=rtl, base=base)

                    def rowterm_update(d):
                        """rowterm = s2 - BC in cols; fp16 split; transpose
                        into rbank rows 1 (hi) and 4 (lo) via rank-1 trick."""
                        svf, svh = d["svf"], d["svh"]
                        nc.vector.tensor_sub(svf[:, AUXC], svf[:, S2C],
                                             svf[:, BCC])
                        nc.vector.tensor_copy(svh[:, 0:8], svf[:, AUXC])
                        nc.vector.tensor_copy(svf[:, LNX], svh[:, 0:8])
                        nc.vector.tensor_sub(svf[:, LNX], svf[:, AUXC],
                                             svf[:, LNX])
                        nc.vector.tensor_copy(svh[:, 8:16], svf[:, LNX])
                        for (cols, dstrow) in ((svh[:, 0:8], d["rth"]),
                                               (svh[:, 8:16], d["rtl"])):
                            for h in range(2):
                                psr = ps_b.tile([128, 512], fp32,
                                                tag="bldps")
                                for c in range(4):
                                    cc = h * 4 + c
                                    nc.tensor.matmul(
                                        psr[0:1, c * 128:(c + 1) * 128],
                                        cols[:, cc:cc + 1], identh[:],
                                        start=True, stop=True)
                                nc.scalar.activation(
                                    dstrow[0:1, h * 512:(h + 1) * 512],
                                    psr[0:1, :], AF.Copy)

                    def pre_psum(d, jc, h, ps):
                        """ps = cross - (s2 - BC) via 2 K-packed fp16 MMs."""
                        ja, jb = jc * 128, (jc + 1) * 128
                        ha, hb = h * 512, (h + 1) * 512
                        nc.tensor.matmul(ps[:], d["lb"][0:3, ja:jb],
                                         d["rb"][0:3, ha:hb],
                                         start=True, stop=False)
                        nc.tensor.matmul(ps[:], neg1h[:],
                                         d["rth"][:, ha:hb],
                                         start=False, stop=False)
                        nc.tensor.matmul(ps[:], neg1h[:],
                                         d["rtl"][:, ha:hb],
                                         start=False, stop=True)

                    # ---- init pass: BRC = -max_k(pre with BC=0) ----
                    for i in gidx:
                        d = st[i]
                        svf = d["svf"]
                        rowterm_update(d)
                        for jc in range(JC):
                            for h in range(2):
                                ps = ps_b.tile([128, 512], fp32, tag="bldps")
                                pre_psum(d, jc, h, ps)
                                nc.vector.tensor_reduce(
                                    out=svf[:, RM][:, h:h + 1], in_=ps[:],
                                    op=ALU.max, axis=AX.X)
                            nc.vector.tensor_max(
                                svf[:, RM][:, 0:1], svf[:, RM][:, 0:1],
                                svf[:, RM][:, 1:2])
                            nc.vector.tensor_scalar_mul(
                                svf[:, BRC][:, jc:jc + 1],
                                svf[:, RM][:, 0:1], -1.0)

                    # ---- segment loop --------------------------------------
                    def side_pair(use_mt, ucol_sel, use_act):
                        """one matvec side for the whole group, emission
                        interleaved so the 2 samples' streams overlap on
                        separate PE col-groups."""
                        pss_ = {}
                        for i in gidx:
                            d = st[i]
                            base = d["base"]
                            mat = d["MT"] if use_mt else d["M"]
                            invec = (d["svb"][:, 8:16] if ucol_sel == "v"
                                     else d["svb"][:, 0:8])
                            pss = ps_i.tile([128, D], fp32, tag="ivps")
                            pss_[i] = pss
                            for h in range(2):
                                for kc in range(JC):
                                    nc.tensor.matmul(
                                        pss[base:base + 1,
                                            h * 512:(h + 1) * 512],
                                        invec[:, kc:kc + 1],
                                        mat[:, kc, h * 512:(h + 1) * 512],
                                        start=(kc == 0),
                                        stop=(kc == JC - 1),
                                        tile_position=(0, base))
                        for i in gidx:
                            d = st[i]
                            if use_act:
                                nc.scalar.activation(
                                    d["srow"][:],
                                    pss_[i][d["base"]:d["base"] + 1, :],
                                    AF.Copy)
                            else:
                                nc.vector.tensor_copy(
                                    d["srow"][:],
                                    pss_[i][d["base"]:d["base"] + 1, :])
                        psc_ = {}
                        for i in gidx:
                            d = st[i]
                            psc = ps_s.tile([128, 512], fp32, tag="smallps")
                            psc_[i] = psc
                            for c in range(JC):
                                nc.tensor.matmul(
                                    psc[:, c:c + 1],
                                    d["srow"][0:1, c * 128:(c + 1) * 128],
                                    oneb[:], start=True, stop=True)
                        for i in gidx:
                            d = st[i]
                            out_cols = d["svf"][:, UCF if ucol_sel == "v"
                                                else VCF]
                            out_colsb = (d["svb"][:, 0:8] if ucol_sel == "v"
                                         else d["svb"][:, 8:16])
                            nc.vector.reciprocal(out_cols[:],
                                                 psc_[i][:, :JC])
                            nc.vector.tensor_copy(out_colsb[:], out_cols[:])

                    def seg_body():
                        for i in gidx:
                            d = st[i]
                            svf, svb = d["svf"], d["svb"]
                            rowterm_update(d)
                            for jc in range(JC):
                                for h in range(2):
                                    ps = ps_b.tile([128, 512], fp32,
                                                   tag="bldps")
                                    pre_psum(d, jc, h, ps)
                                    nc.scalar.activation(
                                        d["M"][:, jc, h * 512:(h + 1) * 512],
                                        ps[:], AF.Exp,
                                        bias=svf[:, BRC][:, jc:jc + 1])
                                for kb in range(2):
                                    pst = ps_s.tile([128, 512], bf16,
                                                    tag="smallps")
                                    for c in range(4):
                                        kc = kb * 4 + c
                                        nc.tensor.transpose(
                                            pst[:, c * 128:(c + 1) * 128],
                                            d["M"][:, jc,
                                                   kc * 128:(kc + 1) * 128],
                                            identb[:])
                                    nc.vector.tensor_copy(
                                        d["MT"][:, kb * 4:(kb + 1) * 4,
                                                jc * 128:(jc + 1) * 128],
                                        pst[:].rearrange(
                                            "p (c q) -> p c q", c=4))
                            nc.gpsimd.memset(svb[:, 8:16], 1.0)  # vcol = 1
                        for t in range(seg_len):
                            side_pair(use_mt=True, ucol_sel="v",
                                      use_act=True)
                            side_pair(use_mt=False, ucol_sel="u",
                                      use_act=False)
                        for i in gidx:
                            svf = st[i]["svf"]
                            nc.scalar.activation(svf[:, LNX], svf[:, UCF],
                                                 AF.Ln)
                            nc.vector.tensor_add(svf[:, BRC], svf[:, BRC],
                                                 svf[:, LNX])
                            nc.scalar.activation(svf[:, LNX], svf[:, VCF],
                                                 AF.Ln)
                            nc.vector.tensor_add(svf[:, BCC], svf[:, BCC],
                                                 svf[:, LNX])

                    if n_seg > 1:
                        with tc.For_i(0, n_seg, 1, hint_engines=(ET.PE,)):
                            seg_body()
                    else:
                        seg_body()

                    # ---- P accumulation: D*P = exp(pre + BRC) --------------
                    for i in gidx:
                        d = st[i]
                        svf = d["svf"]
                        rowterm_update(d)
                        for jc in range(JC):
                            for h in range(2):
                                ps = ps_b.tile([128, 512], fp32, tag="bldps")
                                pre_psum(d, jc, h, ps)
                                ptile = ppool.tile([128, 512], fp32,
                                                   tag="ptile")
                                nc.scalar.activation(
                                    ptile[:], ps[:], AF.Exp,
                                    bias=svf[:, BRC][:, jc:jc + 1])
                                nc.vector.tensor_add(
                                    pacc[:, jc, h * 512:(h + 1) * 512],
                                    pacc[:, jc, h * 512:(h + 1) * 512],
                                    ptile[:])

            # ------------- phase 3: AllReduce + finale ----------------------
            pacc_b = dpool.tile([D, D], fp32)
            pall_b = dpool.tile(
                [D, D], fp32,
                addr_space="Shared" if n_cores > 4 else "Local")
            nc.sync.dma_start(
                pacc_b[:].rearrange("(jc p) k -> p jc k", p=128), pacc[:])
            if skip_collective:
                nc.sync.dma_start(pall_b[:], pacc_b[:])
            else:
                nc.gpsimd.collective_compute(
                    "AllReduce", ALU.add,
                    replica_groups=[list(range(n_cores))],
                    ins=[pacc_b.opt()], outs=[pall_b.opt()],
                )
            with tc.tile_pool(name="fin", bufs=1) as fpool:
                ot = fpool.tile([128, JC, D], fp32)
                nc.sync.dma_start(
                    ot[:], pall_b[:].rearrange("(jc p) k -> p jc k", p=128))
                dl = fpool.tile([128, JC, D], fp32)
                nc.sync.dma_start(
                    dl[:],
                    delta_d.ap().rearrange("(jc p) k -> p jc k", p=128))
                nc.vector.tensor_scalar_mul(ot[:], ot[:], SCALE / N_GLOB)
                nc.vector.tensor_add(ot[:], ot[:], dl[:])
                out_sb = fpool.tile([ns, D], fp32)
                for h in range(2):
                    pso = ps_b.tile([128, 512], fp32, tag="bldps")
                    for jc in range(JC):
                        nc.tensor.matmul(
                            pso[:ns], srccol[:, :, jc],
                            ot[:, jc, h * 512:(h + 1) * 512],
                            start=(jc == 0), stop=(jc == JC - 1))
                    nc.scalar.activation(
                        out_sb[:, h * 512:(h + 1) * 512], pso[:ns], AF.Copy)
                nc.sync.dma_start(out_d.ap()[:], out_sb[:])

    nc.compile()
    return nc


def kernel(**inputs):
    X = np.ascontiguousarray(inputs["X"], np.float32)
    Y = np.ascontiguousarray(inputs["Y"], np.float32)
    W = np.ascontiguousarray(inputs["W"], np.float32)
    b = np.ascontiguousarray(inputs["b"], np.float32).reshape(1, D)
    delta = np.ascontiguousarray(inputs["delta_ot"], np.float32)

    from concourse import bass_utils

    if "nc" not in _cache:
        _cache["nc"] = build()
    nc = _cache["nc"]

    in_maps = []
    for c in range(N_CORES):
        sl = slice(c * NS, (c + 1) * NS)
        in_maps.append({
            "x": X[sl], "y": Y[sl], "w": W, "bvec": b, "delta": delta,
        })
    res = bass_utils.run_bass_kernel_spmd(
        nc, in_maps, core_ids=list(range(N_CORES)))
    out = np.concatenate([res.results[c]["out"] for c in range(N_CORES)],
                         axis=0)
    return out.astype(np.float32)


if __name__ == "__main__":
    import reference
    ins = reference.setup_inputs()
    ins = {k: np.asarray(v) for k, v in ins.items()}
    got = kernel(**ins)
    print("out", got.shape, got.dtype)
